# revision 1
# baseline (speedup 1.0000x reference)
"""Trainium2 Bass kernel for the capsule-routing layer (nn_Caps_Layer).

Computation (per batch b of x [B, S, D], W [D, 25]):
  u_hat = (x_b @ W).reshape(S, 5, 5)           # [S, n, k], col = n*5+k
  b0 = 0;  for 4 routing iters:
    c = softmax_n(b)                            # over the 5 capsules
    v[n,k] = sum_s c[n,s] u_hat[s,n,k]
    out = v / sqrt(sum_k v^2 + 1e-7)
    b[n,s] = sum_k out[n,k] u_hat[s,n,k]
Returns out [B, 5, 5].

Sharding: pure data-parallel over batch across 8 NeuronCores (16 batches
each); W replicated; no collectives.

Per-core pipeline:
  phase 1 (per batch): DMA x rows -> SBUF natural [128s x (4, 768)];
    PE-transpose 128x128 blocks -> xT; matmul W[dblk].T @ xT -> u_hatT
    [25, 512] in PSUM; PE-transpose back -> u_hat natural
    [128 s_lo, (s_hi n k)] gathered per group into UH.
  phase 2 (per group of batches): dynamic routing with
    [128, G*100]-shaped elementwise/reduce ops, partition sums via
    ones-matmul on PE, softmax without max-subtraction (|logits| < ~10).
"""

from contextlib import ExitStack

import numpy as np

import concourse.bass as bass
import concourse.tile as tile
from concourse import mybir, masks

F32 = mybir.dt.float32
F32R = mybir.dt.float32r
BF16 = mybir.dt.bfloat16
AX = mybir.AxisListType
OP = mybir.AluOpType
AF = mybir.ActivationFunctionType

N_CORES = 8
B_FULL, S, D = 128, 512, 768
NCAP, KDIM = 5, 5
NK = NCAP * KDIM  # 25
ROUTINGS = 4
T_EPS = 1e-7

ND = D // 128   # 6 d-blocks
NSB = S // 128  # 4 s-blocks (= s_hi)


def emit(ctx, tc, out, x, w, b_loc=16, group=8, mm_dt=F32, t_dt=F32):
    """Emit the per-core kernel IR.

    out: [1, b_loc*25] f32; x: [b_loc*512, 768] f32; w: [768, 25] f32.
    mm_dt: dtype of the main-matmul inputs (F32 or F32R).
    t_dt: dtype of the routing c*u_hat product feeding partition-sum mms.
    """
    nc = tc.nc
    groups = list(group) if isinstance(group, (list, tuple)) else \
        [group] * (b_loc // group)
    assert sum(groups) == b_loc

    const_pool = ctx.enter_context(tc.tile_pool(name="const", bufs=1))
    xnat_pool = ctx.enter_context(tc.tile_pool(name="xnat", bufs=4))
    ptr_pool = ctx.enter_context(tc.tile_pool(name="ptr", bufs=3, space="PSUM"))
    xt_pool = ctx.enter_context(tc.tile_pool(name="xt", bufs=3))
    pu_pool = ctx.enter_context(tc.tile_pool(name="pu", bufs=2, space="PSUM"))
    uhT_pool = ctx.enter_context(tc.tile_pool(name="uhT", bufs=2))
    uh_pool = ctx.enter_context(tc.tile_pool(name="uh", bufs=3))
    rt_pool = ctx.enter_context(tc.tile_pool(name="rt", bufs=2))
    pv_pool = ctx.enter_context(tc.tile_pool(name="pv", bufs=1, space="PSUM"))

    # --- constants ---
    ident = const_pool.tile([128, 128], F32)
    masks.make_identity(nc, ident[:])
    ident_m = const_pool.tile([128, 128], mm_dt)
    nc.scalar.copy(ident_m[:], ident[:])
    w_raw = const_pool.tile([128, ND * NK], F32)
    # DRAM [768, 25] -> [128, (dblk, nk)]
    nc.sync.dma_start(
        w_raw[:].rearrange("p (nb k) -> p nb k", nb=ND),
        w.rearrange("(nb p) k -> p nb k", p=128),
    )
    w_sb = const_pool.tile([128, ND * NK], mm_dt)
    nc.scalar.copy(w_sb[:], w_raw[:])

    ones_col = const_pool.tile([128, 1], t_dt)
    nc.gpsimd.memset(ones_col[:], 1.0)
    ones_col_f = const_pool.tile([128, 1], F32)
    nc.gpsimd.memset(ones_col_f[:], 1.0)
    ones_row = const_pool.tile([1, 128], F32)
    nc.gpsimd.memset(ones_row[:], 1.0)
    cs_row = const_pool.tile([1, 128], F32)
    nc.gpsimd.memset(cs_row[:], 1.0 / NCAP)

    def warm_pe():
        """Tiny REGULAR matmul: transpose-mode PE activity is invisible to
        the HAM clock gate, so phase 1 otherwise runs at 1.2 GHz. ~110 ns
        of real matmul every few us keeps K=8/8 (2.4 GHz)."""
        wps = pv_pool.tile([1, 128], F32, tag="warm")
        nc.tensor.matmul(wps[:], ones_col_f[:], ident[:], start=True, stop=True)
    eps1 = const_pool.tile([1, 1], F32)
    nc.gpsimd.memset(eps1[:], T_EPS)

    # HAM warm-up: ~5us of back-to-back REGULAR matmuls (transpose-mode
    # activity never flips the clock gate to 8/8). Overlaps the first DMA.
    wps = pv_pool.tile([1, 128], F32, tag="warm")
    for _ in range(24):
        nc.tensor.matmul(wps[:], ones_col_f[:], ident[:], start=True, stop=True)

    b_off = 0
    for g, G in enumerate(groups):
        uh = uh_pool.tile([128, G * NSB * NK], F32, tag="uh")  # [128,(b,s_hi,n,k)]
        for bi in range(G):
            b = b_off + bi
            # --- load x rows for batch b: [512, 768] -> [128, (sblk, d)] ---
            x_nat = xnat_pool.tile([128, NSB * D], mm_dt)
            nc.sync.dma_start(
                x_nat[:].rearrange("p (sb d) -> p sb d", sb=NSB),
                x[b * S:(b + 1) * S, :].rearrange("(sb p) d -> p sb d", p=128),
            )
            # --- transpose to xT [128 d_lo, (dblk, s)]; db-major so each
            # d-block's matmul can issue as soon as its own copy lands ---
            xT = xt_pool.tile([128, ND * S], mm_dt)
            xT3 = xT[:].rearrange("p (db s) -> p db s", db=ND)
            pu = pu_pool.tile([NK, S], F32)
            for db in range(ND):
                ptr = ptr_pool.tile([128, S], mm_dt)
                for sb_i in range(NSB):
                    nc.tensor.transpose(
                        ptr[:, sb_i * 128:(sb_i + 1) * 128],
                        x_nat[:, sb_i * D + db * 128:sb_i * D + (db + 1) * 128],
                        ident_m[:],
                    )
                # copy psum -> sbuf (rounds to f32r when mm_dt is f32r);
                # spread across ACT and DVE to unblock the scalar engine
                if db in (1, 4):
                    nc.vector.tensor_copy(xT3[:, db], ptr[:])
                else:
                    nc.scalar.copy(xT3[:, db], ptr[:])
                nc.tensor.matmul(
                    pu[:],
                    w_sb[:, db * NK:(db + 1) * NK],
                    xT3[:, db],
                    start=(db == 0),
                    stop=(db == ND - 1),
                )
                if db % 2 == 1:
                    warm_pe()
            uhT = uhT_pool.tile([NK, S], F32)
            nc.vector.tensor_copy(uhT[:], pu[:])
            # --- transpose back: u_hat natural [128 s_lo, (s_hi, n, k)] ---
            pnat = ptr_pool.tile([128, NSB * NK], F32, tag="ptr")
            for sh in range(NSB):
                nc.tensor.transpose(
                    pnat[:, sh * NK:(sh + 1) * NK],
                    uhT[:, sh * 128:(sh + 1) * 128],
                    ident[0:NK, 0:NK],
                )
            nc.vector.tensor_copy(
                uh[:, bi * NSB * NK:(bi + 1) * NSB * NK], pnat[:]
            )

        # ---------------- routing for this group ----------------
        # Critical-chain-minimized form. Per iter i:
        #   c   = softmax_n(b)                    (skipped on i=0: c = 1/5)
        #   v   = sum_s c*u_hat                   (PE partition-sum, raw v)
        #   rnrm = (cs^2*|v|^2 + eps)^-1/2        (side branch, 1-partition)
        #   b'  = (sum_k v*u_hat) * (cs*rnrm)     (cs folded into the rnrm
        #                                          broadcast matmul's ones)
        # where cs = 1/5 on iter 0 (from the constant softmax), else 1.
        # Final outputs (iter 3) = v * rnrm on partition 0 only.
        uh_ap = uh[:].rearrange("p (b sh n k) -> p b sh n k", b=G, sh=NSB, n=NCAP)
        blog = rt_pool.tile([128, G * NSB * NCAP], F32, tag="blog")
        for it in range(ROUTINGS):
            cs = 1.0 / NCAP if it == 0 else 1.0
            if it == 0:
                t_ap = uh_ap
            else:
                expb = rt_pool.tile([128, G * NSB * NCAP], F32, tag="expb")
                nc.scalar.activation(expb[:], blog[:], AF.Exp)
                den = rt_pool.tile([128, G * NSB], F32, tag="den")
                nc.vector.reduce_sum(
                    den[:],
                    expb[:].rearrange("p (bs n) -> p bs n", n=NCAP),
                    axis=AX.X,
                )
                rden = rt_pool.tile([128, G * NSB], F32, tag="rden")
                nc.vector.reciprocal(rden[:], den[:])
                c = rt_pool.tile([128, G * NSB * NCAP], F32, tag="c")
                nc.vector.tensor_tensor(
                    c[:].rearrange("p (b sh n) -> p b sh n", b=G, sh=NSB),
                    expb[:].rearrange("p (b sh n) -> p b sh n", b=G, sh=NSB),
                    rden[:].rearrange("p (b sh) -> p b sh", b=G)
                    .unsqueeze(3)
                    .broadcast_to((128, G, NSB, NCAP)),
                    op=OP.mult,
                )
                c_b = (
                    c[:]
                    .rearrange("p (b sh n) -> p b sh n", b=G, sh=NSB)
                    .unsqueeze(4)
                    .broadcast_to((128, G, NSB, NCAP, KDIM))
                )
                t = rt_pool.tile([128, G * NSB * NK], t_dt, tag="t")
                t_ap = t[:].rearrange(
                    "p (b sh n k) -> p b sh n k", b=G, sh=NSB, n=NCAP
                )
                nc.vector.tensor_tensor(t_ap, uh_ap, c_b, op=OP.mult)
            # ---- v[n,k] = sum_s t: partition sum via ones matmul ----
            pv = pv_pool.tile([1, G * NK], F32, tag="pv")
            for sh in range(NSB):
                nc.tensor.matmul(
                    pv[:],
                    ones_col[:],
                    t_ap[:, :, sh, :, :],
                    start=(sh == 0),
                    stop=(sh == NSB - 1),
                )
            warm_pe()
            v_sb = rt_pool.tile([1, G * NK], F32, tag="v_sb")
            nc.vector.tensor_copy(v_sb[:], pv[:])
            # ---- side branch: rnrm = (cs^2*|v|^2 + eps)^-1/2 on part 0 ----
            sq = rt_pool.tile([1, G * NK], F32, tag="sq")
            nc.vector.tensor_tensor(sq[:], v_sb[:], v_sb[:], op=OP.mult)
            s2 = rt_pool.tile([1, G * NCAP], F32, tag="s2")
            nc.vector.reduce_sum(
                s2[:], sq[:].rearrange("p (bn k) -> p bn k", k=KDIM), axis=AX.X
            )
            nrm = rt_pool.tile([1, G * NCAP], F32, tag="nrm")
            nc.scalar.activation(
                nrm[:], s2[:], AF.Sqrt, bias=eps1[:], scale=cs * cs
            )
            rnrm = rt_pool.tile([1, G * NCAP], F32, tag="rnrm")
            nc.vector.reciprocal(rnrm[:], nrm[:])
            if it < ROUTINGS - 1:
                # ---- main chain: w = sum_k v*u_hat via pvb broadcast ----
                pvb = pv_pool.tile([128, G * NK], F32, tag="pvb")
                nc.tensor.matmul(pvb[:], ones_row[:], v_sb[:], start=True, stop=True)
                tmp = rt_pool.tile([128, G * NSB * NK], F32, tag="tmp")
                tmp_ap = tmp[:].rearrange(
                    "p (b sh n k) -> p b sh n k", b=G, sh=NSB, n=NCAP
                )
                nc.vector.tensor_tensor(
                    tmp_ap,
                    uh_ap,
                    pvb[:]
                    .rearrange("p (b n k) -> p b n k", b=G, n=NCAP)
                    .unsqueeze(2)
                    .broadcast_to((128, G, NSB, NCAP, KDIM)),
                    op=OP.mult,
                )
                w_t = rt_pool.tile([128, G * NSB * NCAP], F32, tag="w_t")
                nc.vector.reduce_sum(
                    w_t[:],
                    tmp[:].rearrange("p (bsn k) -> p bsn k", k=KDIM),
                    axis=AX.X,
                )
                # broadcast cs*rnrm to all partitions (cs via the ones value)
                prn = pv_pool.tile([128, G * NCAP], F32, tag="pvb")
                nc.tensor.matmul(
                    prn[:],
                    cs_row[:] if it == 0 else ones_row[:],
                    rnrm[:],
                    start=True,
                    stop=True,
                )
                blog = rt_pool.tile([128, G * NSB * NCAP], F32, tag="blog")
                nc.vector.tensor_tensor(
                    blog[:].rearrange("p (b sh n) -> p b sh n", b=G, sh=NSB),
                    w_t[:].rearrange("p (b sh n) -> p b sh n", b=G, sh=NSB),
                    prn[:]
                    .rearrange("p (b n) -> p b n", b=G)
                    .unsqueeze(2)
                    .broadcast_to((128, G, NSB, NCAP)),
                    op=OP.mult,
                )
            else:
                # ---- final outputs on partition 0 (cs == 1 here) ----
                outputs_sb = rt_pool.tile([1, G * NK], F32, tag="outs")
                nc.vector.tensor_tensor(
                    outputs_sb[:].rearrange("p (b n k) -> p b n k", b=G, n=NCAP),
                    v_sb[:].rearrange("p (b n k) -> p b n k", b=G, n=NCAP),
                    rnrm[:]
                    .rearrange("p (b n) -> p b n", b=G)
                    .unsqueeze(3)
                    .broadcast_to((1, G, NCAP, KDIM)),
                    op=OP.mult,
                )
                nc.sync.dma_start(
                    out[0:1, b_off * NK:(b_off + G) * NK],
                    outputs_sb[0:1, :],
                )
        b_off += G


def legalize_waits(nc):
    """This toolchain's walrus codegen accepts at most ONE sync wait per
    instruction ("Too many sync wait commands" otherwise) — and PE Matmult
    appears to take none safely. Hoist excess waits onto wait-only
    EventSemaphore instructions inserted just before, on the same engine
    (same pattern walrus already accepts for Tile's engine barriers)."""
    n = 0
    for fn in nc.m.functions:
        for blk in fn.blocks:
            new = []
            for inst in blk.instructions:
                si = inst.sync_info
                if si is not None and len(si.on_wait) > 0:
                    waits = list(si.on_wait)
                    keep = 0 if type(inst).__name__ == "InstMatmult" else 1
                    if len(waits) > keep:
                        for wt in waits[: len(waits) - keep]:
                            ev = mybir.InstEventSemaphore(
                                name=f"I-waitfix-{nc.next_id()}"
                            )
                            ev.engine = inst.engine
                            ev.sync_info = mybir.SyncInfo(on_wait=[wt], on_update=[])
                            new.append(ev)
                            n += 1
                        si.on_wait = waits[len(waits) - keep:]
                new.append(inst)
            blk.instructions = new
    return n


def build_caps_kernel(b_loc=16, group=8, mm_dt=F32, t_dt=F32):
    nc = bass.Bass(trn_type="TRN2", debug=False, target_bir_lowering=False)
    x = nc.dram_tensor("x", [b_loc * S, D], mm_dt, kind="ExternalInput").ap()
    w = nc.dram_tensor("w", [D, NK], F32, kind="ExternalInput").ap()
    out = nc.dram_tensor("out", [1, b_loc * NK], F32, kind="ExternalOutput").ap()
    with tile.TileContext(nc) as tc:
        with ExitStack() as ctx:
            emit(ctx, tc, out, x, w, b_loc=b_loc, group=group, mm_dt=mm_dt, t_dt=t_dt)
    legalize_waits(nc)
    return nc


_KERNEL_CFG = dict(group=8, mm_dt=F32R, t_dt=F32)


def kernel(x: np.ndarray, W: np.ndarray) -> np.ndarray:
    from concourse.bass_utils import run_bass_kernel_spmd

    B, S_, D_ = x.shape
    assert (B, S_, D_) == (B_FULL, S, D)
    b_loc = B // N_CORES
    nc = build_caps_kernel(b_loc=b_loc, **_KERNEL_CFG)
    in_maps = [
        {
            "x": np.ascontiguousarray(
                x[i * b_loc:(i + 1) * b_loc].reshape(b_loc * S, D)
            ),
            "w": np.ascontiguousarray(W),
        }
        for i in range(N_CORES)
    ]
    res = run_bass_kernel_spmd(nc, in_maps, core_ids=list(range(N_CORES)))
    outs = [res.results[i]["out"].reshape(b_loc, NCAP, KDIM) for i in range(N_CORES)]
    return np.concatenate(outs, axis=0).astype(np.float32)



# revision 13
# speedup vs baseline: 1.4563x; 1.4563x over previous
"""Trainium2 Bass kernel for the capsule-routing layer (nn_Caps_Layer).

Computation (per batch b of x [B, S, D], W [D, 25]):
  u_hat = (x_b @ W).reshape(S, 5, 5)           # [S, n, k], col = n*5+k
  b0 = 0;  for 4 routing iters:
    c = softmax_n(b)                            # over the 5 capsules
    v[n,k] = sum_s c[n,s] u_hat[s,n,k]
    out = v / sqrt(sum_k v^2 + 1e-7)
    b[n,s] = sum_k out[n,k] u_hat[s,n,k]
Returns out [B, 5, 5].

Sharding: pure data-parallel over batch across 8 NeuronCores (16 batches
each); W replicated; no collectives.

v2 design (memory-roofline oriented):
  - x and W are cast to bf16 on the HOST, halving HBM traffic.
  - xT arrives via the XBAR DMA-transpose (HBM -> SBUF, 2-byte dtype),
    eliminating every PE transpose and PSUM->SBUF staging copy of x.
  - u_hat natural layout comes straight out of the PE: per (s-chunk,
    d-block) mini-matmul with the xT block STATIONARY and the tiny W
    block MOVING (25 cols), accumulating over d-blocks in PSUM.
  - Routing tensors are bf16 with k OUTERMOST in the free axis so the
    broadcast multiplies keep a packed 2-byte last dim (2x DVE mode)
    and the k-sum becomes paired adds.
  - 1/sqrt is exp(-0.5*ln(.)): Exp and Ln live in one ACT table, so the
    kernel never reloads activation tables (Copy/Square are in every
    table).
  - Groups of batches are routed while the next group's phase 1 runs;
    emission is interleaved so the PE never queues routing matmuls ahead
    of phase-1 work it could be doing.
"""

from contextlib import ExitStack

import numpy as np

import concourse.bass as bass
import concourse.tile as tile
from concourse import mybir

F32 = mybir.dt.float32
BF16 = mybir.dt.bfloat16
AX = mybir.AxisListType
OP = mybir.AluOpType
AF = mybir.ActivationFunctionType

N_CORES = 8
B_FULL, S, D = 128, 512, 768
NCAP, KDIM = 5, 5
NK = NCAP * KDIM  # 25
ROUTINGS = 4
T_EPS = 1e-7

ND = D // 128   # 6 d-blocks
NSB = S // 128  # 4 s-blocks


def emit(ctx, tc, out, x, w, b_loc=16, groups=(4, 4, 4, 4)):
    nc = tc.nc
    groups = list(groups)
    assert sum(groups) == b_loc

    const_pool = ctx.enter_context(tc.tile_pool(name="const", bufs=1))
    xt_pool = ctx.enter_context(tc.tile_pool(name="xt", bufs=b_loc))
    pu_pool = ctx.enter_context(tc.tile_pool(name="pu", bufs=2, space="PSUM"))
    uh_pool = ctx.enter_context(tc.tile_pool(name="uh", bufs=2))
    rt_pool = ctx.enter_context(tc.tile_pool(name="rt", bufs=2))
    pv_pool = ctx.enter_context(tc.tile_pool(name="pv", bufs=2, space="PSUM"))

    # --- constants ---
    w_sb = const_pool.tile([128, ND * NK], BF16)
    nc.sync.dma_start(
        w_sb[:].rearrange("p (nb k) -> p nb k", nb=ND),
        w.rearrange("(nb p) k -> p nb k", p=128),
    )
    ones_col = const_pool.tile([128, 1], BF16)
    nc.gpsimd.memset(ones_col[:], 1.0)
    ones_row = const_pool.tile([1, 128], BF16)
    nc.gpsimd.memset(ones_row[:], 1.0)
    eps1 = const_pool.tile([1, 1], F32)
    nc.gpsimd.memset(eps1[:], T_EPS)
    # iteration-0 softmax is uniform: fold c = 1/NCAP into the rsqrt
    # broadcast via exp(-0.5*ln(..) + ln(1/NCAP))
    lcs1 = const_pool.tile([1, 1], F32)
    nc.gpsimd.memset(lcs1[:], float(np.log(1.0 / NCAP)))
    zero1 = const_pool.tile([1, 1], F32)
    nc.gpsimd.memset(zero1[:], 0.0)

    def warm_pe():
        """Tiny bf16 matmul: keeps the HAM clock gate and PE p-state at
        full speed through routing stretches where the PE would idle."""
        wps = pv_pool.tile([1, 64], F32, tag="warm", bufs=1)
        nc.tensor.matmul(wps[:], ones_col[:], w_sb[:, 0:64], start=True, stop=True)

    # --- issue every x transpose-DMA up front; the HWDGE queue drains
    # them in order while compute follows ---
    xts = []
    for b in range(b_loc):
        xt = xt_pool.tile([128, ND * S], BF16, tag="xt", name=f"xt{b}")
        nc.sync.dma_start(
            xt[:].rearrange("p (db s) -> p db s", db=ND),
            x[b * S:(b + 1) * S, :],
            transpose=True,
        )
        xts.append(xt)

    # a few warm matmuls to start the PE p-state ramp under DMA #0
    for _ in range(6):
        warm_pe()

    uh_tiles = {}

    def emit_phase1_batch(g, bi, b):
        """24 mini-matmuls (xT block stationary, W block moving) -> u_hat
        natural [128 s_lo, (sc, n, k)] in PSUM; copy to the group's uh
        tile in (k, b, sh, n) order (bf16)."""
        G = groups[g]
        if bi == 0:
            uh_tiles[g] = uh_pool.tile(
                [128, KDIM * G * NSB * NCAP], BF16, tag="uh", name=f"uh{g}"
            )
        uh5 = uh_tiles[g][:].rearrange(
            "p (k b sh n) -> p k b sh n", k=KDIM, b=G, sh=NSB
        )
        xt3 = xts[b][:].rearrange("p (db s) -> p db s", db=ND)
        pu = pu_pool.tile([128, NSB * NK], F32, tag="pu")
        for sc in range(NSB):
            for db in range(ND):
                nc.tensor.matmul(
                    pu[:, sc * NK:(sc + 1) * NK],
                    xt3[:, db, sc * 128:(sc + 1) * 128],
                    w_sb[:, db * NK:(db + 1) * NK],
                    start=(db == 0),
                    stop=(db == ND - 1),
                )
        nc.scalar.copy(
            uh5[:, :, bi, :, :],
            pu[:].rearrange("p (sh n k) -> p k sh n", sh=NSB, n=NCAP),
        )

    blogs = {}

    def emit_routing_iter(g, it, b_off, chain_mode):
        """One dynamic-routing iteration for group g.

        State tensors (free axis, k outermost where present):
          blog [128, (b, sh, n)] f32     routing logits
          uh   [128, (k, b, sh, n)] bf16
        chain_mode keeps latency-critical ops on the DVE for the final
        group; otherwise the k-sum pair-adds go to the idle Pool engine.
        """
        G = groups[g]
        uh5 = uh_tiles[g][:].rearrange(
            "p (k b sh n) -> p k b sh n", k=KDIM, b=G, sh=NSB
        )
        nbsn = G * NSB * NCAP  # (b, sh, n) block
        if it == 0:
            t5 = uh5
        else:
            blog = blogs[g]
            expb = rt_pool.tile([128, nbsn], F32, tag="expb")
            nc.scalar.activation(expb[:], blog[:], AF.Exp)
            den = rt_pool.tile([128, G * NSB], F32, tag="den")
            nc.vector.reduce_sum(
                den[:],
                expb[:].rearrange("p (bs n) -> p bs n", n=NCAP),
                axis=AX.X,
            )
            rden = rt_pool.tile([128, G * NSB], F32, tag="rden")
            nc.vector.reciprocal(rden[:], den[:])
            c = rt_pool.tile([128, nbsn], BF16, tag="c")
            nc.vector.tensor_tensor(
                c[:].rearrange("p (b sh n) -> p b sh n", b=G, sh=NSB),
                expb[:].rearrange("p (b sh n) -> p b sh n", b=G, sh=NSB),
                rden[:].rearrange("p (b sh) -> p b sh", b=G)
                .unsqueeze(3)
                .broadcast_to((128, G, NSB, NCAP)),
                op=OP.mult,
            )
            t = rt_pool.tile([128, KDIM * nbsn], BF16, tag="t")
            t5 = t[:].rearrange(
                "p (k b sh n) -> p k b sh n", k=KDIM, b=G, sh=NSB
            )
            nc.vector.tensor_tensor(
                t5,
                uh5,
                c[:].rearrange("p (b sh n) -> p b sh n", b=G, sh=NSB)
                .unsqueeze(1)
                .broadcast_to((128, KDIM, G, NSB, NCAP)),
                op=OP.mult,
            )
        # ---- raw v[(k, b, n)] = sum_s t via ones-matmul partition sum ----
        pv = pv_pool.tile([1, KDIM * G * NCAP], F32, tag="pv")
        for sh in range(NSB):
            nc.tensor.matmul(
                pv[:],
                ones_col[:],
                t5[:, :, :, sh, :],
                start=(sh == 0),
                stop=(sh == NSB - 1),
            )
        warm_pe()
        # ---- side branch: rnrm = exp(-0.5*ln(cs^2*|v|^2 + eps) [+ ln cs])
        # (one ACT table serves Exp, Ln, Copy, Square: never reloads) ----
        cs = 1.0 / NCAP if it == 0 else 1.0
        sq = rt_pool.tile([1, KDIM * G * NCAP], F32, tag="sq")
        nc.scalar.activation(sq[:], pv[:], AF.Square)
        s2 = rt_pool.tile([1, G * NCAP], F32, tag="s2")
        nc.vector.reduce_sum(
            s2[:],
            sq[:].rearrange("o (k bn) -> o bn k", k=KDIM),
            axis=AX.X,
        )
        lnv = rt_pool.tile([1, G * NCAP], F32, tag="lnv")
        nc.scalar.activation(lnv[:], s2[:], AF.Ln, bias=eps1[:], scale=cs * cs)
        if it < ROUTINGS - 1:
            rnrm = rt_pool.tile([1, G * NCAP], BF16, tag="rnrm")
            nc.scalar.activation(
                rnrm[:], lnv[:], AF.Exp,
                bias=lcs1[:] if it == 0 else zero1[:], scale=-0.5,
            )
            # ---- main chain: w = sum_k v*u_hat ----
            v_sb = rt_pool.tile([1, KDIM * G * NCAP], BF16, tag="v_sb")
            nc.scalar.copy(v_sb[:], pv[:])
            pvb = pv_pool.tile([128, KDIM * G * NCAP], F32, tag="pvb")
            nc.tensor.matmul(pvb[:], ones_row[:], v_sb[:], start=True, stop=True)
            pvb_sb = rt_pool.tile([128, KDIM * G * NCAP], BF16, tag="pvb_sb")
            nc.scalar.copy(pvb_sb[:], pvb[:])
            tmp = rt_pool.tile([128, KDIM * nbsn], BF16, tag="tmp")
            nc.vector.tensor_tensor(
                tmp[:].rearrange(
                    "p (k b sh n) -> p k b sh n", k=KDIM, b=G, sh=NSB
                ),
                uh5,
                pvb_sb[:].rearrange("p (k b n) -> p k b n", k=KDIM, b=G)
                .unsqueeze(3)
                .broadcast_to((128, KDIM, G, NSB, NCAP)),
                op=OP.mult,
            )
            # k-sum as paired adds (packed bf16 2x on DVE; Pool offload in
            # throughput mode)
            eng = nc.vector  # TODO: gpsimd offload rejected bf16 TT? bisecting
            tk = [tmp[:, k * nbsn:(k + 1) * nbsn] for k in range(KDIM)]
            wa = rt_pool.tile([128, nbsn], BF16, tag="wa")
            eng.tensor_tensor(wa[:], tk[0], tk[1], op=OP.add)
            wb = rt_pool.tile([128, nbsn], BF16, tag="wb")
            eng.tensor_tensor(wb[:], tk[2], tk[3], op=OP.add)
            wc = rt_pool.tile([128, nbsn], BF16, tag="wc")
            eng.tensor_tensor(wc[:], wa[:], wb[:], op=OP.add)
            w_t = rt_pool.tile([128, nbsn], F32, tag="w_t")
            eng.tensor_tensor(w_t[:], wc[:], tk[4], op=OP.add)
            # broadcast rnrm to all partitions, then logits
            prn = pv_pool.tile([128, G * NCAP], F32, tag="prn", bufs=1)
            nc.tensor.matmul(prn[:], ones_row[:], rnrm[:], start=True, stop=True)
            blog = rt_pool.tile([128, nbsn], F32, tag="blog")
            nc.vector.tensor_tensor(
                blog[:].rearrange("p (b sh n) -> p b sh n", b=G, sh=NSB),
                w_t[:].rearrange("p (b sh n) -> p b sh n", b=G, sh=NSB),
                prn[:].rearrange("p (b n) -> p b n", b=G)
                .unsqueeze(2)
                .broadcast_to((128, G, NSB, NCAP)),
                op=OP.mult,
            )
            blogs[g] = blog
        else:
            # ---- final outputs on partition 0 (cs == 1 here) ----
            rnrm_f = rt_pool.tile([1, G * NCAP], F32, tag="rnrm_f")
            nc.scalar.activation(rnrm_f[:], lnv[:], AF.Exp, scale=-0.5)
            outs = rt_pool.tile([1, G * NK], F32, tag="outs")
            nc.vector.tensor_tensor(
                outs[:].rearrange("o (b n k) -> o k b n", n=NCAP, k=KDIM),
                pv[:].rearrange("o (k b n) -> o k b n", k=KDIM, b=G),
                rnrm_f[:].rearrange("o (b n) -> o b n", b=G)
                .unsqueeze(1)
                .broadcast_to((1, KDIM, G, NCAP)),
                op=OP.mult,
            )
            nc.sync.dma_start(
                out[0:1, b_off * NK:(b_off + G) * NK], outs[0:1, :]
            )

    # ---- interleaved schedule: phase1(g0); then routing(g) woven between
    # phase1 batches of g+1; routing of the last group trails alone ----
    offs = [sum(groups[:i]) for i in range(len(groups))]
    for bi in range(groups[0]):
        emit_phase1_batch(0, bi, bi)
    for g in range(len(groups)):
        nxt = (
            [(g + 1, bi, offs[g + 1] + bi) for bi in range(groups[g + 1])]
            if g + 1 < len(groups) else []
        )
        iters = list(range(ROUTINGS))
        last = g + 1 >= len(groups)
        while nxt or iters:
            if nxt:
                emit_phase1_batch(*nxt.pop(0))
            if iters:
                emit_routing_iter(g, iters.pop(0), offs[g], chain_mode=last)


def legalize_waits(nc):
    """This toolchain's walrus codegen accepts at most ONE sync wait per
    instruction ("Too many sync wait commands" otherwise) — and PE Matmult
    appears to take none safely. Hoist excess waits onto wait-only
    EventSemaphore instructions inserted just before, on the same engine
    (same pattern walrus already accepts for Tile's engine barriers)."""
    n = 0
    for fn in nc.m.functions:
        for blk in fn.blocks:
            new = []
            for inst in blk.instructions:
                si = inst.sync_info
                if si is not None and len(si.on_wait) > 0:
                    waits = list(si.on_wait)
                    keep = 0 if type(inst).__name__ == "InstMatmult" else 1
                    if len(waits) > keep:
                        for wt in waits[: len(waits) - keep]:
                            ev = mybir.InstEventSemaphore(
                                name=f"I-waitfix-{nc.next_id()}"
                            )
                            ev.engine = inst.engine
                            ev.sync_info = mybir.SyncInfo(on_wait=[wt], on_update=[])
                            new.append(ev)
                            n += 1
                        si.on_wait = waits[len(waits) - keep:]
                new.append(inst)
            blk.instructions = new
    return n


def build_caps_kernel(b_loc=16, groups=(4, 4, 4, 4)):
    nc = bass.Bass(trn_type="TRN2", debug=False, target_bir_lowering=False)
    x = nc.dram_tensor("x", [b_loc * S, D], BF16, kind="ExternalInput").ap()
    w = nc.dram_tensor("w", [D, NK], BF16, kind="ExternalInput").ap()
    out = nc.dram_tensor("out", [1, b_loc * NK], F32, kind="ExternalOutput").ap()
    with nc.allow_low_precision(reason="bf16 5-term k-sums; f32 final add"):
        with tile.TileContext(nc) as tc:
            with ExitStack() as ctx:
                emit(ctx, tc, out, x, w, b_loc=b_loc, groups=groups)
    legalize_waits(nc)
    return nc


_KERNEL_CFG = dict(groups=(4, 4, 4, 4))


def make_inmaps(x: np.ndarray, W: np.ndarray, b_loc: int):
    import ml_dtypes

    xb = x.astype(ml_dtypes.bfloat16)
    wb = np.ascontiguousarray(W.astype(ml_dtypes.bfloat16))
    return [
        {
            "x": np.ascontiguousarray(
                xb[i * b_loc:(i + 1) * b_loc].reshape(b_loc * S, D)
            ),
            "w": wb,
        }
        for i in range(N_CORES)
    ]


def kernel(x: np.ndarray, W: np.ndarray) -> np.ndarray:
    from concourse.bass_utils import run_bass_kernel_spmd

    B, S_, D_ = x.shape
    assert (B, S_, D_) == (B_FULL, S, D)
    b_loc = B // N_CORES
    nc = build_caps_kernel(b_loc=b_loc, **_KERNEL_CFG)
    in_maps = make_inmaps(x, W, b_loc)
    res = run_bass_kernel_spmd(nc, in_maps, core_ids=list(range(N_CORES)))
    outs = [res.results[i]["out"].reshape(b_loc, NCAP, KDIM) for i in range(N_CORES)]
    return np.concatenate(outs, axis=0).astype(np.float32)


# revision 16
# speedup vs baseline: 1.5388x; 1.0566x over previous
"""Trainium2 Bass kernel for the capsule-routing layer (nn_Caps_Layer).

Computation (per batch b of x [B, S, D], W [D, 25]):
  u_hat = (x_b @ W).reshape(S, 5, 5)           # [S, n, k], col = n*5+k
  b0 = 0;  for 4 routing iters:
    c = softmax_n(b)                            # over the 5 capsules
    v[n,k] = sum_s c[n,s] u_hat[s,n,k]
    out = v / sqrt(sum_k v^2 + 1e-7)
    b[n,s] = sum_k out[n,k] u_hat[s,n,k]
Returns out [B, 5, 5].

Sharding: pure data-parallel over batch across 8 NeuronCores (16 batches
each); W replicated; no collectives.

v3 design:
  - x and W cast to FLOAT16 on the host (fp16, not bf16: the routing
    iteration chaotically amplifies u_hat perturbations; bf16's 2^-8
    ulp costs ~1.6e-2 rel error while fp16 keeps it ~4e-3) — halves HBM
    traffic and keeps the XBAR DMA-transpose (2-byte only) usable.
  - xT arrives via XBAR DMA-transpose (HBM -> SBUF), eliminating every
    PE transpose and staging copy of x. Transposes alternate between
    the two HWDGE queues (sync / scalar).
  - u_hat natural layout straight out of the PE: per (s-chunk, d-block)
    matmul with the xT block STATIONARY and the tiny W block MOVING,
    accumulating over d-blocks in PSUM.
  - Routing free-axis layout (k, b, sh, n): broadcast multiplies keep a
    packed 2-byte last dim (2x DVE) and k-sums become paired adds.
  - One ones-matmul per iteration (moving = whole t tile, <=512 cols)
    plus a DVE sh-reduce replaces 4 serial accumulating matmuls; v then
    lives in SBUF f32 (no extra copy, exact squares).
  - Routing state v / rnrm / w_t kept in f32 (f32r for PE broadcasts):
    per-iteration rounding noise injections were the accuracy killers.
  - 1/sqrt = exp(-0.5*ln(.)): Exp and Ln share one ACT table with Copy/
    Square, so the activation table never reloads.
  - softmax normalize via a single divide op.
  - Routing for two groups is emitted op-interleaved (generators), so
    the in-order engines ping-pong between two independent dependency
    chains instead of idling on one; the first pair hides under the
    second pair's phase 1.
"""

from contextlib import ExitStack

import numpy as np

import concourse.bass as bass
import concourse.tile as tile
from concourse import mybir

F32 = mybir.dt.float32
F32R = mybir.dt.float32r
F16 = mybir.dt.float16
AX = mybir.AxisListType
OP = mybir.AluOpType
AF = mybir.ActivationFunctionType

N_CORES = 8
B_FULL, S, D = 128, 512, 768
NCAP, KDIM = 5, 5
NK = NCAP * KDIM  # 25
ROUTINGS = 4
T_EPS = 1e-7

ND = D // 128   # 6 d-blocks
NSB = S // 128  # 4 s-blocks


def emit(ctx, tc, out, x, w, b_loc=16, groups=(4, 4, 4, 4), dual_q=True):
    nc = tc.nc
    groups = list(groups)
    ngr = len(groups)
    assert sum(groups) == b_loc
    assert all(KDIM * g * NSB * NCAP <= 512 for g in groups), "pv matmul >512 cols"

    const_pool = ctx.enter_context(tc.tile_pool(name="const", bufs=1))
    xt_pool = ctx.enter_context(tc.tile_pool(name="xt", bufs=b_loc))
    pu_pool = ctx.enter_context(tc.tile_pool(name="pu", bufs=2, space="PSUM"))
    uh_pool = ctx.enter_context(tc.tile_pool(name="uh", bufs=2))
    rt_pool = ctx.enter_context(tc.tile_pool(name="rt", bufs=2))
    pv_pool = ctx.enter_context(tc.tile_pool(name="pv", bufs=2, space="PSUM"))

    # --- constants ---
    w_sb = const_pool.tile([128, ND * NK], F16)
    nc.sync.dma_start(
        w_sb[:].rearrange("p (nb k) -> p nb k", nb=ND),
        w.rearrange("(nb p) k -> p nb k", p=128),
    )
    ones_col = const_pool.tile([128, 1], F16)
    nc.gpsimd.memset(ones_col[:], 1.0)
    ones_row = const_pool.tile([1, 128], F32)
    nc.gpsimd.memset(ones_row[:], 1.0)
    eps1 = const_pool.tile([1, 1], F32)
    nc.gpsimd.memset(eps1[:], T_EPS)
    # iteration-0 softmax is uniform: fold c = 1/NCAP into the rsqrt
    # broadcast via exp(-0.5*ln(..) + ln(1/NCAP))
    lcs1 = const_pool.tile([1, 1], F32)
    nc.gpsimd.memset(lcs1[:], float(np.log(1.0 / NCAP)))
    zero1 = const_pool.tile([1, 1], F32)
    nc.gpsimd.memset(zero1[:], 0.0)
    outs_all = const_pool.tile([1, b_loc * NK], F32)

    def warm_pe():
        """Tiny fp16 matmul: keeps the HAM clock gate and PE p-state up
        through routing stretches where the PE would otherwise idle."""
        wps = pv_pool.tile([1, 64], F32, tag="warm", bufs=1)
        nc.tensor.matmul(wps[:], ones_col[:], w_sb[:, 0:64], start=True, stop=True)

    # --- issue every x transpose-DMA up front, alternating HWDGE queues;
    # the DMA rings drain them while compute follows ---
    xts = []
    for b in range(b_loc):
        xt = xt_pool.tile([128, ND * S], F16, tag="xt", name=f"xt{b}")
        eng = nc.sync if (b % 2 == 0 or not dual_q) else nc.scalar
        eng.dma_start(
            xt[:].rearrange("p (db s) -> p db s", db=ND),
            x[b * S:(b + 1) * S, :],
            transpose=True,
        )
        xts.append(xt)

    for _ in range(6):
        warm_pe()

    uh_tiles = {}

    def emit_phase1_batch(g, bi, b):
        """24 mini-matmuls (xT block stationary, W block moving) -> u_hat
        natural [128 s_lo, (sc, n, k)] in PSUM; copy into the group's uh
        tile in (k, b, sh, n) order (fp16)."""
        G = groups[g]
        if bi == 0:
            uh_tiles[g] = uh_pool.tile(
                [128, KDIM * G * NSB * NCAP], F16, tag="uh", name=f"uh{g}"
            )
        uh5 = uh_tiles[g][:].rearrange(
            "p (k b sh n) -> p k b sh n", k=KDIM, b=G, sh=NSB
        )
        xt3 = xts[b][:].rearrange("p (db s) -> p db s", db=ND)
        pu = pu_pool.tile([128, NSB * NK], F32, tag="pu")
        for sc in range(NSB):
            for db in range(ND):
                nc.tensor.matmul(
                    pu[:, sc * NK:(sc + 1) * NK],
                    xt3[:, db, sc * 128:(sc + 1) * 128],
                    w_sb[:, db * NK:(db + 1) * NK],
                    start=(db == 0),
                    stop=(db == ND - 1),
                )
        nc.scalar.copy(
            uh5[:, :, bi, :, :],
            pu[:].rearrange("p (sh n k) -> p k sh n", sh=NSB, n=NCAP),
        )

    blogs = {}

    def routing_iter_steps(g, it, b_off, chain_mode):
        """Generator emitting one routing iteration for group g, yielding
        between steps so two groups' chains can be op-interleaved.

        Free-axis layouts: blog [*, (b, sh, n)] f32, uh/t/tmp
        [*, (k, b, sh, n)] fp16, v/pv [*, (k, b, n)] f32.
        chain_mode=True keeps every op on the fast engines (last pair);
        otherwise bulk k-sums go to the idle Pool engine.
        """
        G = groups[g]
        uh5 = uh_tiles[g][:].rearrange(
            "p (k b sh n) -> p k b sh n", k=KDIM, b=G, sh=NSB
        )
        nbsn = G * NSB * NCAP
        nkbn = KDIM * G * NCAP
        if it == 0:
            t_mv = uh_tiles[g][:]
        else:
            blog = blogs[g]
            expb = rt_pool.tile([128, nbsn], F32, tag="expb")
            nc.scalar.activation(expb[:], blog[:], AF.Exp)
            yield
            den = rt_pool.tile([128, G * NSB], F32, tag="den")
            nc.vector.reduce_sum(
                den[:],
                expb[:].rearrange("p (bs n) -> p bs n", n=NCAP),
                axis=AX.X,
            )
            yield
            rden = rt_pool.tile([128, G * NSB], F32, tag="rden")
            nc.vector.reciprocal(rden[:], den[:])
            yield
            c = rt_pool.tile([128, nbsn], F16, tag="c")
            nc.vector.tensor_tensor(
                c[:].rearrange("p (b sh n) -> p b sh n", b=G, sh=NSB),
                expb[:].rearrange("p (b sh n) -> p b sh n", b=G, sh=NSB),
                rden[:].rearrange("p (b sh) -> p b sh", b=G)
                .unsqueeze(3)
                .broadcast_to((128, G, NSB, NCAP)),
                op=OP.mult,
            )
            yield
            t = rt_pool.tile([128, KDIM * nbsn], F16, tag="t")
            nc.vector.tensor_tensor(
                t[:].rearrange("p (k b sh n) -> p k b sh n", k=KDIM, b=G, sh=NSB),
                uh5,
                c[:].rearrange("p (b sh n) -> p b sh n", b=G, sh=NSB)
                .unsqueeze(1)
                .broadcast_to((128, KDIM, G, NSB, NCAP)),
                op=OP.mult,
            )
            t_mv = t[:]
            yield
        # ---- raw v[(k, b, n)] = sum_s t: one ones-matmul (partition sum,
        # sh stays in the free axis) + DVE sh-reduce -> v in SBUF f32 ----
        pv = pv_pool.tile([1, KDIM * nbsn], F32, tag="pv")
        nc.tensor.matmul(pv[:], ones_col[:], t_mv, start=True, stop=True)
        if it % 2 == 0:
            warm_pe()
        yield
        v = rt_pool.tile([1, nkbn], F32, tag="v")
        nc.vector.reduce_sum(
            v[:],
            pv[:].rearrange("o (k b sh n) -> o k b n sh", k=KDIM, b=G, sh=NSB),
            axis=AX.X,
        )
        yield
        # ---- side branch: rnrm = exp(-0.5*ln(cs^2*|v|^2 + eps) [+ ln cs])
        # (Exp/Ln/Copy/Square share one ACT table: never reloads) ----
        cs = 1.0 / NCAP if it == 0 else 1.0
        sq = rt_pool.tile([1, nkbn], F32, tag="sq")
        nc.vector.tensor_tensor(sq[:], v[:], v[:], op=OP.mult)
        yield
        s2 = rt_pool.tile([1, G * NCAP], F32, tag="s2")
        nc.vector.reduce_sum(
            s2[:],
            sq[:].rearrange("o (k b n) -> o b n k", k=KDIM, b=G),
            axis=AX.X,
        )
        yield
        lnv = rt_pool.tile([1, G * NCAP], F32, tag="lnv")
        nc.scalar.activation(lnv[:], s2[:], AF.Ln, bias=eps1[:], scale=cs * cs)
        yield
        if it < ROUTINGS - 1:
            rnrm = rt_pool.tile([1, G * NCAP], F32, tag="rnrm")
            nc.scalar.activation(
                rnrm[:], lnv[:], AF.Exp,
                bias=lcs1[:] if it == 0 else zero1[:], scale=-0.5,
            )
            yield
            # ---- main chain: w = sum_k v*u_hat, logits = w * rnrm ----
            pvb = pv_pool.tile([128, nkbn], F32, tag="pvb")
            nc.tensor.matmul(pvb[:], ones_row[:], v[:], start=True, stop=True)
            yield
            tmp = rt_pool.tile([128, KDIM * nbsn], F16, tag="tmp")
            nc.vector.tensor_tensor(
                tmp[:].rearrange(
                    "p (k b sh n) -> p k b sh n", k=KDIM, b=G, sh=NSB
                ),
                uh5,
                pvb[:].rearrange("p (k b n) -> p k b n", k=KDIM, b=G)
                .unsqueeze(3)
                .broadcast_to((128, KDIM, G, NSB, NCAP)),
                op=OP.mult,
            )
            yield
            eng = nc.vector if chain_mode else nc.gpsimd
            tk = [tmp[:, k * nbsn:(k + 1) * nbsn] for k in range(KDIM)]
            wa = rt_pool.tile([128, nbsn], F16, tag="wa")
            eng.tensor_tensor(wa[:], tk[0], tk[1], op=OP.add)
            wb = rt_pool.tile([128, nbsn], F16, tag="wb")
            eng.tensor_tensor(wb[:], tk[2], tk[3], op=OP.add)
            yield
            wc = rt_pool.tile([128, nbsn], F16, tag="wc")
            eng.tensor_tensor(wc[:], wa[:], wb[:], op=OP.add)
            wt = rt_pool.tile([128, nbsn], F32, tag="wt")
            eng.tensor_tensor(wt[:], wc[:], tk[4], op=OP.add)
            yield
            prn = pv_pool.tile([128, G * NCAP], F32, tag="prn", bufs=1)
            nc.tensor.matmul(prn[:], ones_row[:], rnrm[:], start=True, stop=True)
            yield
            blog = rt_pool.tile([128, nbsn], F32, tag="blog")
            nc.vector.tensor_tensor(
                blog[:].rearrange("p (b sh n) -> p b sh n", b=G, sh=NSB),
                wt[:].rearrange("p (b sh n) -> p b sh n", b=G, sh=NSB),
                prn[:].rearrange("p (b n) -> p b n", b=G)
                .unsqueeze(2)
                .broadcast_to((128, G, NSB, NCAP)),
                op=OP.mult,
            )
            blogs[g] = blog
            yield
        else:
            # ---- final outputs (cs == 1): v * rnrm into the gather tile ----
            rnrm_f = rt_pool.tile([1, G * NCAP], F32, tag="rnrm_f")
            nc.scalar.activation(rnrm_f[:], lnv[:], AF.Exp, scale=-0.5)
            yield
            nc.vector.tensor_tensor(
                outs_all[0:1, b_off * NK:(b_off + G) * NK]
                .rearrange("o (b n k) -> o k b n", n=NCAP, k=KDIM),
                v[:].rearrange("o (k b n) -> o k b n", k=KDIM, b=G),
                rnrm_f[:].rearrange("o (b n) -> o b n", b=G)
                .unsqueeze(1)
                .broadcast_to((1, KDIM, G, NCAP)),
                op=OP.mult,
            )
            yield

    def pair_steps(ga, gb, it, offs, chain_mode):
        """Op-interleave one iteration of two independent groups."""
        gens = [routing_iter_steps(ga, it, offs[ga], chain_mode)]
        if gb is not None:
            gens.append(routing_iter_steps(gb, it, offs[gb], chain_mode))
        alive = [True] * len(gens)
        while any(alive):
            for i, gen in enumerate(gens):
                if alive[i]:
                    try:
                        next(gen)
                    except StopIteration:
                        alive[i] = False

    # ---- schedule ----
    offs = [sum(groups[:i]) for i in range(ngr)]
    pairs = [(i, i + 1 if i + 1 < ngr else None) for i in range(0, ngr, 2)]

    # phase 1 of the first pair
    for g in pairs[0][:2]:
        if g is None:
            continue
        for bi in range(groups[g]):
            emit_phase1_batch(g, bi, offs[g] + bi)

    for pi, (ga, gb) in enumerate(pairs):
        last = pi + 1 >= len(pairs)
        # batches of the NEXT pair, to weave between this pair's iters
        nxt = []
        if not last:
            for g in pairs[pi + 1][:2]:
                if g is None:
                    continue
                nxt += [(g, bi, offs[g] + bi) for bi in range(groups[g])]
        per_iter = (len(nxt) + ROUTINGS - 1) // ROUTINGS if nxt else 0
        for it in range(ROUTINGS):
            for _ in range(per_iter):
                if nxt:
                    emit_phase1_batch(*nxt.pop(0))
            pair_steps(ga, gb, it, offs, chain_mode=last)
        while nxt:
            emit_phase1_batch(*nxt.pop(0))

    nc.sync.dma_start(out[0:1, :], outs_all[0:1, :])


def legalize_waits(nc):
    """This toolchain's walrus codegen accepts at most ONE sync wait per
    instruction ("Too many sync wait commands" otherwise) — and PE Matmult
    appears to take none safely. Hoist excess waits onto wait-only
    EventSemaphore instructions inserted just before, on the same engine
    (same pattern walrus already accepts for Tile's engine barriers)."""
    n = 0
    for fn in nc.m.functions:
        for blk in fn.blocks:
            new = []
            for inst in blk.instructions:
                si = inst.sync_info
                if si is not None and len(si.on_wait) > 0:
                    waits = list(si.on_wait)
                    keep = 0 if type(inst).__name__ == "InstMatmult" else 1
                    if len(waits) > keep:
                        for wt in waits[: len(waits) - keep]:
                            ev = mybir.InstEventSemaphore(
                                name=f"I-waitfix-{nc.next_id()}"
                            )
                            ev.engine = inst.engine
                            ev.sync_info = mybir.SyncInfo(on_wait=[wt], on_update=[])
                            new.append(ev)
                            n += 1
                        si.on_wait = waits[len(waits) - keep:]
                new.append(inst)
            blk.instructions = new
    return n


def build_caps_kernel(b_loc=16, groups=(4, 4, 4, 4), dual_q=True):
    nc = bass.Bass(trn_type="TRN2", debug=False, target_bir_lowering=False)
    x = nc.dram_tensor("x", [b_loc * S, D], F16, kind="ExternalInput").ap()
    w = nc.dram_tensor("w", [D, NK], F16, kind="ExternalInput").ap()
    out = nc.dram_tensor("out", [1, b_loc * NK], F32, kind="ExternalOutput").ap()
    with nc.allow_low_precision(reason="fp16 k-sums; f32 state"):
        with tile.TileContext(nc) as tc:
            with ExitStack() as ctx:
                emit(ctx, tc, out, x, w, b_loc=b_loc, groups=groups, dual_q=dual_q)
    legalize_waits(nc)
    return nc


# dual_q=True races concurrent XBAR transposes from the two HWDGE queues
# and corrupts the loads (measured rel err ~1.4) — keep single-queue.
_KERNEL_CFG = dict(groups=(4, 4, 4, 4), dual_q=False)


def make_inmaps(x: np.ndarray, W: np.ndarray, b_loc: int):
    xh = x.astype(np.float16)
    wh = np.ascontiguousarray(W.astype(np.float16))
    return [
        {
            "x": np.ascontiguousarray(
                xh[i * b_loc:(i + 1) * b_loc].reshape(b_loc * S, D)
            ),
            "w": wh,
        }
        for i in range(N_CORES)
    ]


def kernel(x: np.ndarray, W: np.ndarray) -> np.ndarray:
    from concourse.bass_utils import run_bass_kernel_spmd

    B, S_, D_ = x.shape
    assert (B, S_, D_) == (B_FULL, S, D)
    b_loc = B // N_CORES
    nc = build_caps_kernel(b_loc=b_loc, **_KERNEL_CFG)
    in_maps = make_inmaps(x, W, b_loc)
    res = run_bass_kernel_spmd(nc, in_maps, core_ids=list(range(N_CORES)))
    outs = [res.results[i]["out"].reshape(b_loc, NCAP, KDIM) for i in range(N_CORES)]
    return np.concatenate(outs, axis=0).astype(np.float32)


# revision 18
# speedup vs baseline: 1.6928x; 1.1001x over previous
"""Trainium2 Bass kernel for the capsule-routing layer (nn_Caps_Layer).

Computation (per batch b of x [B, S, D], W [D, 25]):
  u_hat = (x_b @ W).reshape(S, 5, 5)           # [S, n, k], col = n*5+k
  b0 = 0;  for 4 routing iters:
    c = softmax_n(b)                            # over the 5 capsules
    v[n,k] = sum_s c[n,s] u_hat[s,n,k]
    out = v / sqrt(sum_k v^2 + 1e-7)
    b[n,s] = sum_k out[n,k] u_hat[s,n,k]
Returns out [B, 5, 5].

Sharding: pure data-parallel over batch across 8 NeuronCores (16 batches
each); W replicated; no collectives.

v3 design:
  - x and W cast to FLOAT16 on the host (fp16, not bf16: the routing
    iteration chaotically amplifies u_hat perturbations; bf16's 2^-8
    ulp costs ~1.6e-2 rel error while fp16 keeps it ~4e-3) — halves HBM
    traffic and keeps the XBAR DMA-transpose (2-byte only) usable.
  - xT arrives via XBAR DMA-transpose (HBM -> SBUF), eliminating every
    PE transpose and staging copy of x. Transposes alternate between
    the two HWDGE queues (sync / scalar).
  - u_hat natural layout straight out of the PE: per (s-chunk, d-block)
    matmul with the xT block STATIONARY and the tiny W block MOVING,
    accumulating over d-blocks in PSUM.
  - Routing free-axis layout (k, b, sh, n): broadcast multiplies keep a
    packed 2-byte last dim (2x DVE) and k-sums become paired adds.
  - One ones-matmul per iteration (moving = whole t tile, <=512 cols)
    plus a DVE sh-reduce replaces 4 serial accumulating matmuls; v then
    lives in SBUF f32 (no extra copy, exact squares).
  - Routing state v / rnrm / w_t kept in f32 (f32r for PE broadcasts):
    per-iteration rounding noise injections were the accuracy killers.
  - 1/sqrt = exp(-0.5*ln(.)): Exp and Ln share one ACT table with Copy/
    Square, so the activation table never reloads.
  - softmax normalize via a single divide op.
  - Routing for two groups is emitted op-interleaved (generators), so
    the in-order engines ping-pong between two independent dependency
    chains instead of idling on one; the first pair hides under the
    second pair's phase 1.
"""

from contextlib import ExitStack

import numpy as np

import concourse.bass as bass
import concourse.tile as tile
from concourse import mybir

F32 = mybir.dt.float32
F32R = mybir.dt.float32r
F16 = mybir.dt.float16
AX = mybir.AxisListType
OP = mybir.AluOpType
AF = mybir.ActivationFunctionType

N_CORES = 8
B_FULL, S, D = 128, 512, 768
NCAP, KDIM = 5, 5
NK = NCAP * KDIM  # 25
ROUTINGS = 4
T_EPS = 1e-7

ND = D // 128   # 6 d-blocks
NSB = S // 128  # 4 s-blocks


def emit(ctx, tc, out, x, w, b_loc=16, groups=(4, 4, 4, 4), dual_q=True):
    nc = tc.nc
    groups = list(groups)
    ngr = len(groups)
    assert sum(groups) == b_loc
    assert all(KDIM * g * NSB * NCAP <= 512 for g in groups), "pv matmul >512 cols"

    const_pool = ctx.enter_context(tc.tile_pool(name="const", bufs=1))
    xt_pool = ctx.enter_context(tc.tile_pool(name="xt", bufs=b_loc))
    pu_pool = ctx.enter_context(tc.tile_pool(name="pu", bufs=2, space="PSUM"))
    uh_pool = ctx.enter_context(tc.tile_pool(name="uh", bufs=2))
    rt_pool = ctx.enter_context(tc.tile_pool(name="rt", bufs=2))
    pv_pool = ctx.enter_context(tc.tile_pool(name="pv", bufs=2, space="PSUM"))

    # --- x transpose-DMAs first: the sync queue must not sit behind
    # anything (the XBAR stream is the pacing resource) ---
    xts = []
    for b in range(b_loc):
        xt = xt_pool.tile([128, ND * S], F16, tag="xt", name=f"xt{b}")
        eng = nc.sync if (b % 2 == 0 or not dual_q) else nc.scalar
        eng.dma_start(
            xt[:].rearrange("p (db s) -> p db s", db=ND),
            x[b * S:(b + 1) * S, :],
            transpose=True,
        )
        xts.append(xt)

    # --- constants (W is host-prearranged to [128, (db, nk)]: one clean
    # 300B-per-partition DMA on the scalar HWDGE queue) ---
    w_sb = const_pool.tile([128, ND * NK], F16)
    nc.scalar.dma_start(w_sb[:], w[:, :])
    ones_col = const_pool.tile([128, 1], F16)
    nc.gpsimd.memset(ones_col[:], 1.0)
    ones_row = const_pool.tile([1, 128], F32)
    nc.gpsimd.memset(ones_row[:], 1.0)
    eps1 = const_pool.tile([1, 1], F32)
    nc.gpsimd.memset(eps1[:], T_EPS)
    # iteration-0 softmax is uniform: fold c = 1/NCAP into the rsqrt
    # broadcast via exp(-0.5*ln(..) + ln(1/NCAP))
    lcs1 = const_pool.tile([1, 1], F32)
    nc.gpsimd.memset(lcs1[:], float(np.log(1.0 / NCAP)))
    zero1 = const_pool.tile([1, 1], F32)
    nc.gpsimd.memset(zero1[:], 0.0)
    outs_all = const_pool.tile([1, b_loc * NK], F32)

    def warm_pe():
        """Tiny fp16 matmul: keeps the HAM clock gate and PE p-state up
        through routing stretches where the PE would otherwise idle."""
        wps = pv_pool.tile([1, 64], F32, tag="warm", bufs=1)
        nc.tensor.matmul(wps[:], ones_col[:], w_sb[:, 0:64], start=True, stop=True)

    for _ in range(6):
        warm_pe()

    uh_tiles = {}

    def emit_phase1_batch(g, bi, b):
        """24 mini-matmuls (xT block stationary, W block moving) -> u_hat
        natural [128 s_lo, (sc, n, k)] in PSUM; copy into the group's uh
        tile in (k, b, sh, n) order (fp16)."""
        G = groups[g]
        if bi == 0:
            uh_tiles[g] = uh_pool.tile(
                [128, KDIM * G * NSB * NCAP], F16, tag="uh", name=f"uh{g}"
            )
        uh5 = uh_tiles[g][:].rearrange(
            "p (k b sh n) -> p k b sh n", k=KDIM, b=G, sh=NSB
        )
        xt3 = xts[b][:].rearrange("p (db s) -> p db s", db=ND)
        pu = pu_pool.tile([128, NSB * NK], F32, tag="pu")
        for sc in range(NSB):
            for db in range(ND):
                nc.tensor.matmul(
                    pu[:, sc * NK:(sc + 1) * NK],
                    xt3[:, db, sc * 128:(sc + 1) * 128],
                    w_sb[:, db * NK:(db + 1) * NK],
                    start=(db == 0),
                    stop=(db == ND - 1),
                )
        nc.scalar.copy(
            uh5[:, :, bi, :, :],
            pu[:].rearrange("p (sh n k) -> p k sh n", sh=NSB, n=NCAP),
        )

    blogs = {}

    def routing_iter_steps(g, it, b_off, chain_mode):
        """Generator emitting one routing iteration for group g, yielding
        between steps so two groups' chains can be op-interleaved.

        Free-axis layouts: blog [*, (b, sh, n)] f32, uh/t/tmp
        [*, (k, b, sh, n)] fp16, v/pv [*, (k, b, n)] f32.
        chain_mode=True keeps every op on the fast engines (last pair);
        otherwise bulk k-sums go to the idle Pool engine.
        """
        G = groups[g]
        uh5 = uh_tiles[g][:].rearrange(
            "p (k b sh n) -> p k b sh n", k=KDIM, b=G, sh=NSB
        )
        nbsn = G * NSB * NCAP
        nkbn = KDIM * G * NCAP
        if it == 0:
            t_mv = uh_tiles[g][:]
        else:
            blog = blogs[g]
            expb = rt_pool.tile([128, nbsn], F32, tag="expb")
            nc.scalar.activation(expb[:], blog[:], AF.Exp)
            yield
            den = rt_pool.tile([128, G * NSB], F32, tag="den")
            nc.vector.reduce_sum(
                den[:],
                expb[:].rearrange("p (bs n) -> p bs n", n=NCAP),
                axis=AX.X,
            )
            yield
            rden = rt_pool.tile([128, G * NSB], F32, tag="rden")
            nc.vector.reciprocal(rden[:], den[:])
            yield
            c = rt_pool.tile([128, nbsn], F16, tag="c")
            nc.vector.tensor_tensor(
                c[:].rearrange("p (b sh n) -> p b sh n", b=G, sh=NSB),
                expb[:].rearrange("p (b sh n) -> p b sh n", b=G, sh=NSB),
                rden[:].rearrange("p (b sh) -> p b sh", b=G)
                .unsqueeze(3)
                .broadcast_to((128, G, NSB, NCAP)),
                op=OP.mult,
            )
            yield
            t = rt_pool.tile([128, KDIM * nbsn], F16, tag="t")
            nc.vector.tensor_tensor(
                t[:].rearrange("p (k b sh n) -> p k b sh n", k=KDIM, b=G, sh=NSB),
                uh5,
                c[:].rearrange("p (b sh n) -> p b sh n", b=G, sh=NSB)
                .unsqueeze(1)
                .broadcast_to((128, KDIM, G, NSB, NCAP)),
                op=OP.mult,
            )
            t_mv = t[:]
            yield
        # ---- raw v[(k, b, n)] = sum_s t: one ones-matmul (partition sum,
        # sh stays in the free axis) + DVE sh-reduce -> v in SBUF f32 ----
        pv = pv_pool.tile([1, KDIM * nbsn], F32, tag="pv")
        nc.tensor.matmul(pv[:], ones_col[:], t_mv, start=True, stop=True)
        if it % 2 == 0:
            warm_pe()
        yield
        v = rt_pool.tile([1, nkbn], F32, tag="v")
        nc.vector.reduce_sum(
            v[:],
            pv[:].rearrange("o (k b sh n) -> o k b n sh", k=KDIM, b=G, sh=NSB),
            axis=AX.X,
        )
        yield
        # ---- side branch: rnrm = exp(-0.5*ln(cs^2*|v|^2 + eps) [+ ln cs])
        # (Exp/Ln/Copy/Square share one ACT table: never reloads) ----
        cs = 1.0 / NCAP if it == 0 else 1.0
        sq = rt_pool.tile([1, nkbn], F32, tag="sq")
        nc.gpsimd.tensor_tensor(sq[:], v[:], v[:], op=OP.mult)
        yield
        s2 = rt_pool.tile([1, G * NCAP], F32, tag="s2")
        nc.vector.reduce_sum(
            s2[:],
            sq[:].rearrange("o (k b n) -> o b n k", k=KDIM, b=G),
            axis=AX.X,
        )
        yield
        lnv = rt_pool.tile([1, G * NCAP], F32, tag="lnv")
        nc.scalar.activation(lnv[:], s2[:], AF.Ln, bias=eps1[:], scale=cs * cs)
        yield
        if it < ROUTINGS - 1:
            rnrm = rt_pool.tile([1, G * NCAP], F32, tag="rnrm")
            nc.scalar.activation(
                rnrm[:], lnv[:], AF.Exp,
                bias=lcs1[:] if it == 0 else zero1[:], scale=-0.5,
            )
            yield
            # ---- main chain: w = sum_k v*u_hat, logits = w * rnrm ----
            pvb = pv_pool.tile([128, nkbn], F32, tag="pvb")
            nc.tensor.matmul(pvb[:], ones_row[:], v[:], start=True, stop=True)
            yield
            pvb_sb = rt_pool.tile([128, nkbn], F16, tag="pvb_sb")
            nc.scalar.copy(pvb_sb[:], pvb[:])
            yield
            tmp = rt_pool.tile([128, KDIM * nbsn], F16, tag="tmp")
            nc.vector.tensor_tensor(
                tmp[:].rearrange(
                    "p (k b sh n) -> p k b sh n", k=KDIM, b=G, sh=NSB
                ),
                uh5,
                pvb_sb[:].rearrange("p (k b n) -> p k b n", k=KDIM, b=G)
                .unsqueeze(3)
                .broadcast_to((128, KDIM, G, NSB, NCAP)),
                op=OP.mult,
            )
            yield
            tk = [tmp[:, k * nbsn:(k + 1) * nbsn] for k in range(KDIM)]
            wa = rt_pool.tile([128, nbsn], F16, tag="wa")
            nc.gpsimd.tensor_tensor(wa[:], tk[0], tk[1], op=OP.add)
            wb = rt_pool.tile([128, nbsn], F16, tag="wb")
            nc.vector.tensor_tensor(wb[:], tk[2], tk[3], op=OP.add)
            yield
            wc = rt_pool.tile([128, nbsn], F16, tag="wc")
            nc.vector.tensor_tensor(wc[:], wa[:], wb[:], op=OP.add)
            wt = rt_pool.tile([128, nbsn], F32, tag="wt")
            nc.vector.tensor_tensor(wt[:], wc[:], tk[4], op=OP.add)
            yield
            prn = pv_pool.tile([128, G * NCAP], F32, tag="prn", bufs=1)
            nc.tensor.matmul(prn[:], ones_row[:], rnrm[:], start=True, stop=True)
            yield
            blog = rt_pool.tile([128, nbsn], F32, tag="blog")
            nc.vector.tensor_tensor(
                blog[:].rearrange("p (b sh n) -> p b sh n", b=G, sh=NSB),
                wt[:].rearrange("p (b sh n) -> p b sh n", b=G, sh=NSB),
                prn[:].rearrange("p (b n) -> p b n", b=G)
                .unsqueeze(2)
                .broadcast_to((128, G, NSB, NCAP)),
                op=OP.mult,
            )
            blogs[g] = blog
            yield
        else:
            # ---- final outputs (cs == 1): v * rnrm into the gather tile ----
            rnrm_f = rt_pool.tile([1, G * NCAP], F32, tag="rnrm_f")
            nc.scalar.activation(rnrm_f[:], lnv[:], AF.Exp, scale=-0.5)
            yield
            nc.vector.tensor_tensor(
                outs_all[0:1, b_off * NK:(b_off + G) * NK]
                .rearrange("o (b n k) -> o k b n", n=NCAP, k=KDIM),
                v[:].rearrange("o (k b n) -> o k b n", k=KDIM, b=G),
                rnrm_f[:].rearrange("o (b n) -> o b n", b=G)
                .unsqueeze(1)
                .broadcast_to((1, KDIM, G, NCAP)),
                op=OP.mult,
            )
            yield

    def pair_steps(ga, gb, it, offs, chain_mode):
        """Op-interleave one iteration of two independent groups."""
        gens = [routing_iter_steps(ga, it, offs[ga], chain_mode)]
        if gb is not None:
            gens.append(routing_iter_steps(gb, it, offs[gb], chain_mode))
        alive = [True] * len(gens)
        while any(alive):
            for i, gen in enumerate(gens):
                if alive[i]:
                    try:
                        next(gen)
                    except StopIteration:
                        alive[i] = False

    # ---- schedule ----
    offs = [sum(groups[:i]) for i in range(ngr)]
    pairs = [(i, i + 1 if i + 1 < ngr else None) for i in range(0, ngr, 2)]

    # phase 1 of the first pair
    for g in pairs[0][:2]:
        if g is None:
            continue
        for bi in range(groups[g]):
            emit_phase1_batch(g, bi, offs[g] + bi)

    for pi, (ga, gb) in enumerate(pairs):
        last = pi + 1 >= len(pairs)
        # batches of the NEXT pair, to weave between this pair's iters
        nxt = []
        if not last:
            for g in pairs[pi + 1][:2]:
                if g is None:
                    continue
                nxt += [(g, bi, offs[g] + bi) for bi in range(groups[g])]
        per_iter = (len(nxt) + ROUTINGS - 1) // ROUTINGS if nxt else 0
        for it in range(ROUTINGS):
            for _ in range(per_iter):
                if nxt:
                    emit_phase1_batch(*nxt.pop(0))
            pair_steps(ga, gb, it, offs, chain_mode=last)
        while nxt:
            emit_phase1_batch(*nxt.pop(0))

    nc.sync.dma_start(out[0:1, :], outs_all[0:1, :])


def legalize_waits(nc):
    """This toolchain's walrus codegen accepts at most ONE sync wait per
    instruction ("Too many sync wait commands" otherwise) — and PE Matmult
    appears to take none safely. Hoist excess waits onto wait-only
    EventSemaphore instructions inserted just before, on the same engine
    (same pattern walrus already accepts for Tile's engine barriers)."""
    n = 0
    for fn in nc.m.functions:
        for blk in fn.blocks:
            new = []
            for inst in blk.instructions:
                si = inst.sync_info
                if si is not None and len(si.on_wait) > 0:
                    waits = list(si.on_wait)
                    keep = 0 if type(inst).__name__ == "InstMatmult" else 1
                    if len(waits) > keep:
                        for wt in waits[: len(waits) - keep]:
                            ev = mybir.InstEventSemaphore(
                                name=f"I-waitfix-{nc.next_id()}"
                            )
                            ev.engine = inst.engine
                            ev.sync_info = mybir.SyncInfo(on_wait=[wt], on_update=[])
                            new.append(ev)
                            n += 1
                        si.on_wait = waits[len(waits) - keep:]
                new.append(inst)
            blk.instructions = new
    return n


def build_caps_kernel(b_loc=16, groups=(4, 4, 4, 4), dual_q=True):
    nc = bass.Bass(trn_type="TRN2", debug=False, target_bir_lowering=False)
    x = nc.dram_tensor("x", [b_loc * S, D], F16, kind="ExternalInput").ap()
    w = nc.dram_tensor("w", [128, ND * NK], F16, kind="ExternalInput").ap()
    out = nc.dram_tensor("out", [1, b_loc * NK], F32, kind="ExternalOutput").ap()
    with nc.allow_low_precision(reason="fp16 k-sums; f32 state"):
        with tile.TileContext(nc) as tc:
            with ExitStack() as ctx:
                emit(ctx, tc, out, x, w, b_loc=b_loc, groups=groups, dual_q=dual_q)
    legalize_waits(nc)
    return nc


# dual_q=True races concurrent XBAR transposes from the two HWDGE queues
# and corrupts the loads (measured rel err ~1.4) — keep single-queue.
_KERNEL_CFG = dict(groups=(4, 4, 4, 4), dual_q=False)


def make_inmaps(x: np.ndarray, W: np.ndarray, b_loc: int):
    xh = x.astype(np.float16)
    # pre-arrange W to the on-chip layout [128 d_lo, (d_block, nk)]
    wh = np.ascontiguousarray(
        W.reshape(ND, 128, NK).transpose(1, 0, 2).reshape(128, ND * NK)
        .astype(np.float16)
    )
    return [
        {
            "x": np.ascontiguousarray(
                xh[i * b_loc:(i + 1) * b_loc].reshape(b_loc * S, D)
            ),
            "w": wh,
        }
        for i in range(N_CORES)
    ]


def kernel(x: np.ndarray, W: np.ndarray) -> np.ndarray:
    from concourse.bass_utils import run_bass_kernel_spmd

    B, S_, D_ = x.shape
    assert (B, S_, D_) == (B_FULL, S, D)
    b_loc = B // N_CORES
    nc = build_caps_kernel(b_loc=b_loc, **_KERNEL_CFG)
    in_maps = make_inmaps(x, W, b_loc)
    res = run_bass_kernel_spmd(nc, in_maps, core_ids=list(range(N_CORES)))
    outs = [res.results[i]["out"].reshape(b_loc, NCAP, KDIM) for i in range(N_CORES)]
    return np.concatenate(outs, axis=0).astype(np.float32)
